# revision 27
# baseline (speedup 1.0000x reference)
"""AutoDisBucketEncoder Trainium2 kernel (8 NeuronCores, feature-sharded).

Math (per feature f, batch b):
  h = lrelu(x_aug @ w1_aug)            # bias folded via ones column
  h = lrelu(h @ (rw_l + I) + rb_l)     # x3, residual folded into weights
  z = lrelu(h @ w2 + b2)
  e = exp(z * tau)
  out = (e / sum_k e) @ emb

Layout: features sharded 32/core; each core packs 2 features per 128
partitions (block-diagonal weights), streams the full 2048 batch as the
matmul moving dim.  Softmax runs in [k, b] layout; the sum-over-k and its
broadcast back to 128 partitions are done by one ones-block matmul.

Perf structure (v2):
  * leaky relu runs as Prelu (parametric_relu) -- it lives in the same ACT
    table set as Exp and Copy, so the scalar engine never reloads tables.
  * every PSUM->SBUF evacuation (h evictions, output casts, z epilogue) is
    greedily load-balanced between the scalar (ACT) and vector (DVE)
    engines; DVE uses a custom fused max(x+b,(x+b)*a) op.
  * z matmuls issue q-major so the four col-tiled (M=32) matmuls run
    concurrently in the PE array; emb matmuls issue row-tiled (K=16 at
    row 32j) so four run concurrently per 128-batch block, producing one
    [128,1024] PSUM tile that maps to a contiguous 2KB-per-row output DMA.
"""

import sys

sys.path.insert(0, "/opt/trn_rl_repo")

import numpy as np
import ml_dtypes
from contextlib import ExitStack

BF16 = ml_dtypes.bfloat16
B, F, D, K, E = 2048, 256, 64, 8, 128
NCORES = 8
FC = F // NCORES          # 32 features per core
NPAIR = FC // 2           # 16
NSTACK = NPAIR // 4       # 4 stacks of 4 pairs
NEG = 0.01                # leaky slope
HB = B // 2               # 1024 batch half-chunk (2 PSUM banks in f32)

# build-time engine cost estimates (us) for the greedy ACT/DVE balancer
ACT_COST = 1.12
DVE_COST = 1.26
CP_ACT_COST = 1.12
CP_DVE_COST = 1.22
GPSIMD_TAIL = True  # run softmax mul on the idle Pool engine

_compiled = None
SIM_SAFE = False  # substitute Relu for Prelu so CoreSim can execute


def _register_leaky_bias():
    import numpy as np
    from concourse.dve_spec import Spec, Src0, C0, C1, maxx, lower
    from concourse.dve_ops import (
        DveOp, DveOpSpec, OPS, CUSTOM_DVE_SPECS, _SUB_OPCODE_FOR_NAME,
        _CUSTOM_DVE_ROW_BASE, has_src1,
    )

    if "LEAKY_BIAS_ANT" in CUSTOM_DVE_SPECS:
        return next(o for o in OPS if o.name == "LEAKY_BIAS_ANT")
    spec = Spec(
        body=maxx(Src0 + C0, (Src0 + C0) * C1),
        reference=lambda in0, in1, s0, s1, imm2: np.maximum(
            in0 + s0, (in0 + s0) * s1
        ).astype(np.float32),
    )
    row = _CUSTOM_DVE_ROW_BASE + len(OPS)
    shas = {}
    for ver in ("v3", "v4"):
        uops = lower(spec, ver=ver)
        shas[ver] = DveOpSpec(
            name="LEAKY_BIAS_ANT", opcode=row, uops=uops, rd1_en=has_src1(spec)
        ).sha(ver)
    op = DveOp("LEAKY_BIAS_ANT", spec, subdim=False, uops_sha=shas)
    OPS.append(op)
    CUSTOM_DVE_SPECS[op.name] = spec
    _SUB_OPCODE_FOR_NAME[op.name] = row
    return op


def _build_bass():
    import concourse.bass as bass  # noqa: F401
    import concourse.mybir as mybir
    import concourse.tile as tile
    from concourse import bacc

    LEAKY_OP = _register_leaky_bias()

    dt = mybir.dt
    AF = mybir.ActivationFunctionType
    PRELU = AF.Relu if SIM_SAFE else AF.Prelu

    nc = bacc.Bacc("TRN2", target_bir_lowering=False, debug=False)

    xp = nc.dram_tensor("xp", [NSTACK, 8, 2 * 4 * HB], dt.bfloat16, kind="ExternalInput").ap()
    w1p = nc.dram_tensor("w1p", [8, NPAIR * 128], dt.bfloat16, kind="ExternalInput").ap()
    rwp = nc.dram_tensor("rwp", [128, 3 * NPAIR * 128], dt.bfloat16, kind="ExternalInput").ap()
    rbp = nc.dram_tensor("rbp", [128, 3 * NPAIR], dt.float32, kind="ExternalInput").ap()
    w2p = nc.dram_tensor("w2p", [128, NPAIR * 32], dt.bfloat16, kind="ExternalInput").ap()
    b2s = nc.dram_tensor("b2s", [128, NSTACK], dt.float32, kind="ExternalInput").ap()
    taus = nc.dram_tensor("taus", [128, NSTACK], dt.float32, kind="ExternalInput").ap()
    onesbd = nc.dram_tensor("onesbd", [128, 128], dt.bfloat16, kind="ExternalInput").ap()
    embs = nc.dram_tensor("embs", [128, NSTACK * 256], dt.bfloat16, kind="ExternalInput").ap()
    out = nc.dram_tensor("out", [B, FC * E], dt.bfloat16, kind="ExternalOutput").ap()

    with tile.TileContext(nc) as tc, ExitStack() as ctx:
        const = ctx.enter_context(tc.tile_pool(name="const", bufs=1))
        xpool = ctx.enter_context(tc.tile_pool(name="xpool", bufs=3))
        hpool = ctx.enter_context(tc.tile_pool(name="hpool", bufs=8))
        tpool = ctx.enter_context(tc.tile_pool(name="tpool", bufs=3))
        epool = ctx.enter_context(tc.tile_pool(name="epool", bufs=2))
        rpool = ctx.enter_context(tc.tile_pool(name="rpool", bufs=2))
        opool = ctx.enter_context(tc.tile_pool(name="opool", bufs=4))
        h_ps = ctx.enter_context(tc.tile_pool(name="h_ps", bufs=3, space="PSUM"))
        zs_ps = ctx.enter_context(tc.tile_pool(name="zs_ps", bufs=1, space="PSUM"))

        # ---- constants into SBUF, ordered by first use so the single DMA
        # queue does not delay the first matmuls: w1 + first-chunk x go
        # first, the heavy rw stack next, everything else after ----
        xtiles = {}

        def fetch_x(s_, c_):
            if (s_, c_) in xtiles:
                return
            x_sb = xpool.tile([8, 4 * HB], dt.bfloat16, tag="x", name=f"x{s_}_{c_}")
            nc.sync.dma_start(
                out=x_sb, in_=xp[s_][:, c_ * 4 * HB : (c_ + 1) * 4 * HB]
            )
            xtiles[(s_, c_)] = x_sb

        w1_sb = const.tile([8, NPAIR * 128], dt.bfloat16)
        nc.sync.dma_start(out=w1_sb, in_=w1p)
        fetch_x(0, 0)
        rb_sb = const.tile([128, 3 * NPAIR], dt.float32)
        nc.sync.dma_start(out=rb_sb, in_=rbp)
        rw_sb = const.tile([128, 3 * NPAIR * 128], dt.bfloat16)
        for l in range(3):
            nc.sync.dma_start(
                out=rw_sb[:, l * NPAIR * 128 : (l + 1) * NPAIR * 128],
                in_=rwp[:, l * NPAIR * 128 : (l + 1) * NPAIR * 128],
            )
        fetch_x(0, 1)
        w2_sb = const.tile([128, NPAIR * 32], dt.bfloat16)
        nc.sync.dma_start(out=w2_sb, in_=w2p)
        b2_sb = const.tile([128, NSTACK], dt.float32)
        nc.sync.dma_start(out=b2_sb, in_=b2s)
        tau_sb = const.tile([128, NSTACK], dt.float32)
        nc.sync.dma_start(out=tau_sb, in_=taus)
        ones_sb = const.tile([128, 128], dt.bfloat16)
        nc.sync.dma_start(out=ones_sb, in_=onesbd)
        emb_sb = const.tile([128, NSTACK * 256], dt.bfloat16)
        nc.sync.dma_start(out=emb_sb, in_=embs)

        # out[b, fc*E] viewed as [qb(4), s(4), p(128), i(4), j(4), e(256)]
        out_r = out.rearrange("(qb i p) (s j e) -> qb s p i j e", p=128, i=4, j=4, e=256)

        # ---- greedy ACT/DVE load balancer for PSUM evacuations ----
        load = {"act": 0.0, "dve": 0.0}

        def evict_h(h, ph, rb_ap):
            """psum -> sbuf bf16 with (optional bias add and) leaky relu."""
            if load["act"] + ACT_COST <= load["dve"] + DVE_COST:
                load["act"] += ACT_COST
                if rb_ap is None:
                    nc.scalar.activation(h, ph, PRELU, alpha=NEG)
                else:
                    nc.scalar.activation(h, ph, PRELU, bias=rb_ap, alpha=NEG)
            else:
                load["dve"] += DVE_COST
                nc.vector._custom_dve(
                    LEAKY_OP,
                    out=h,
                    in0=ph,
                    s0=0.0 if rb_ap is None else rb_ap,
                    s1=NEG,
                )

        def copy_out(ob, po):
            """psum f32 -> sbuf bf16 plain cast."""
            if load["act"] + CP_ACT_COST <= load["dve"] + CP_DVE_COST:
                load["act"] += CP_ACT_COST
                nc.scalar.copy(ob, po)
            else:
                load["dve"] += CP_DVE_COST
                nc.vector.tensor_copy(ob, po)

        pending_zt = []    # deferred z-epilogue head (t1 + exp)
        pending_sum = []   # deferred sum-over-k matmuls
        pending_tail = []  # deferred softmax tail (recip/cast/mul)
        pending_emb = []   # emb batch-block closures

        def emit(lst, n=99):
            for _ in range(min(n, len(lst))):
                lst.pop(0)()

        chunks = [(s, c) for s in range(NSTACK) for c in range(2)]
        e_sbs = {}
        for ci, (s, c) in enumerate(chunks):
            if c == 0:
                e_sbs[s] = (
                    epool.tile([128, B], dt.bfloat16, tag="e", name=f"e{s}"),
                    epool.tile([128, B], dt.bfloat16, tag="en", name=f"en{s}"),
                )
            e_sb, en_sb = e_sbs[s]
            if True:
                # ---- h pipeline: pairs interleaved layer-step-wise; the
                # previous chunk's softmax/emb work drips in between steps ----
                emit(pending_zt)
                fetch_x(s, c)
                x_sb = xtiles.pop((s, c))
                hs = [x_sb[:, j * HB : (j + 1) * HB] for j in range(4)]
                if ci + 1 < len(chunks):
                    fetch_x(*chunks[ci + 1])
                for step in range(4):  # 0: L1, 1-3: residual layers
                    for j in range(4):
                        p = 4 * s + j
                        ph = h_ps.tile(
                            [128, HB], dt.float32, tag="h", name=f"ph{p}_{c}_{step}"
                        )
                        if step == 0:
                            wsl = w1_sb[:, p * 128 : (p + 1) * 128]
                            rb_ap = None
                        else:
                            l = step - 1
                            wsl = rw_sb[
                                :, (l * NPAIR + p) * 128 : (l * NPAIR + p + 1) * 128
                            ]
                            rb_ap = rb_sb[:, l * NPAIR + p : l * NPAIR + p + 1]
                        for q in range(2):
                            nc.tensor.matmul(
                                ph[:, q * 512 : (q + 1) * 512],
                                wsl,
                                hs[j][:, q * 512 : (q + 1) * 512],
                                start=True,
                                stop=True,
                            )
                        h2 = hpool.tile(
                            [128, HB], dt.bfloat16, tag="h", name=f"h{p}_{c}_{step}"
                        )
                        evict_h(h2, ph, rb_ap)
                        hs[j] = h2
                        if step >= 2:
                            emit(pending_emb, 1)
                    if step == 0:
                        emit(pending_sum)
                    elif step == 1:
                        emit(pending_tail)
                # z matmuls: q-major so the 4 col-tiled (M=32) mms overlap
                pz = zs_ps.tile([128, HB], dt.float32, tag="zs", name=f"pz{s}_{c}")
                for q in range(2):
                    for j in range(4):
                        p = 4 * s + j
                        nc.tensor.matmul(
                            pz[32 * j : 32 * j + 32, q * 512 : (q + 1) * 512],
                            w2_sb[:, p * 32 : (p + 1) * 32],
                            hs[j][:, q * 512 : (q + 1) * 512],
                            start=True,
                            stop=True,
                            tile_position=(0, 32 * j),
                        )
                emit(pending_emb)

                def make_z(s_, c_, pz_ref, e_ref, en_ref):
                    def z_head():
                        t1 = tpool.tile(
                            [128, HB], dt.float32, tag="zt", name=f"t1_{s_}_{c_}"
                        )
                        b2_ap = b2_sb[:, s_ : s_ + 1]
                        if load["act"] + ACT_COST <= load["dve"] + DVE_COST:
                            load["act"] += ACT_COST
                            nc.scalar.activation(
                                t1, pz_ref, PRELU, bias=b2_ap, alpha=NEG
                            )
                        else:
                            load["dve"] += DVE_COST
                            nc.vector._custom_dve(
                                LEAKY_OP, out=t1, in0=pz_ref, s0=b2_ap, s1=NEG
                            )
                        ev = e_ref[:, c_ * HB : (c_ + 1) * HB]
                        load["act"] += ACT_COST
                        nc.scalar.activation(
                            ev, t1, AF.Exp, scale=tau_sb[:, s_ : s_ + 1]
                        )

                        def z_sum():
                            ps_sum = zs_ps.tile(
                                [128, HB], dt.float32, tag="zs", name=f"psum{s_}_{c_}"
                            )
                            for q in range(2):
                                nc.tensor.matmul(
                                    ps_sum[:, q * 512 : (q + 1) * 512],
                                    ones_sb,
                                    ev[:, q * 512 : (q + 1) * 512],
                                    start=True,
                                    stop=True,
                                )

                            def tail():
                                rcf = rpool.tile(
                                    [128, HB], dt.float32, tag="rcf",
                                    name=f"rcf{s_}_{c_}",
                                )
                                load["dve"] += DVE_COST
                                nc.vector.reciprocal_approx_fast(out=rcf, in_=ps_sum)
                                rc = rpool.tile(
                                    [128, HB], dt.bfloat16, tag="rc",
                                    name=f"rc{s_}_{c_}",
                                )
                                en_full = en_ref[:, c_ * HB : (c_ + 1) * HB]
                                load["dve"] += 0.83
                                nc.vector.tensor_copy(rc, rcf)
                                for hh in range(2):
                                    if GPSIMD_TAIL:
                                        nc.gpsimd.tensor_mul(
                                            en_full[:, hh * 512 : (hh + 1) * 512],
                                            ev[:, hh * 512 : (hh + 1) * 512],
                                            rc[:, hh * 512 : (hh + 1) * 512],
                                        )
                                    else:
                                        load["dve"] += 0.45
                                        nc.vector.tensor_mul(
                                            en_full[:, hh * 512 : (hh + 1) * 512],
                                            ev[:, hh * 512 : (hh + 1) * 512],
                                            rc[:, hh * 512 : (hh + 1) * 512],
                                        )

                            pending_tail.append(tail)

                        pending_sum.append(z_sum)

                    return z_head

                pending_zt.append(make_z(s, c, pz, e_sb, en_sb))

                def make_group(s_, qb_, j_, en_ref):
                    def emit_group():
                        po = h_ps.tile(
                            [128, 4, 256],
                            dt.float32,
                            tag="h",
                            name=f"po{s_}_{qb_}_{j_}",
                        )
                        for i in range(4):
                            bc2 = qb_ * 4 + i
                            nc.tensor.matmul(
                                po[:, i, :],
                                en_ref[
                                    32 * j_ : 32 * j_ + 16, bc2 * 128 : (bc2 + 1) * 128
                                ],
                                emb_sb[32 * j_ : 32 * j_ + 16, s_ * 256 : (s_ + 1) * 256],
                                start=True,
                                stop=True,
                                tile_position=(32 * j_, 0),
                            )
                        ob = opool.tile(
                            [128, 4, 256], dt.bfloat16, tag="o",
                            name=f"ob{s_}_{qb_}_{j_}",
                        )
                        copy_out(ob, po)
                        nc.sync.dma_start(out=out_r[qb_, s_][:, :, j_, :], in_=ob)

                    return emit_group

                for qb in (2 * c, 2 * c + 1):
                    for j in range(4):
                        pending_emb.append(make_group(s, qb, j, en_sb))
        emit(pending_zt)
        emit(pending_sum)
        emit(pending_tail)
        emit(pending_emb)

    nc.compile()
    return nc


def _host_pack(inputs):
    """Pack full f32 inputs into per-core bf16 device arrays."""
    x = np.ascontiguousarray(inputs["x"], dtype=np.float32)
    w1 = np.asarray(inputs["w1"], dtype=np.float32)
    b1 = np.asarray(inputs["b1"], dtype=np.float32)
    w2 = np.asarray(inputs["w2"], dtype=np.float32)
    b2 = np.asarray(inputs["b2"], dtype=np.float32)
    tau = np.asarray(inputs["tau"], dtype=np.float32)
    emb = np.asarray(inputs["emb"], dtype=np.float32)
    rws = [np.asarray(inputs[f"rw{l}"], dtype=np.float32) for l in range(3)]
    rbs = [np.asarray(inputs[f"rb{l}"], dtype=np.float32) for l in range(3)]

    eye = np.eye(D, dtype=np.float32)
    xT = np.concatenate([x, np.ones((B, F, 1), np.float32)], axis=2)
    xT = np.ascontiguousarray(xT.transpose(1, 2, 0))  # [F, 4, B]
    w1a = np.concatenate([w1, b1[:, None, :]], axis=1)  # [F, 4, D]

    in_maps = []
    for cidx in range(NCORES):
        f0 = cidx * FC
        xpk = np.zeros((NPAIR, 8, B), BF16)
        xq = np.zeros((NSTACK, 8, 2, 4, HB), BF16)
        w1k = np.zeros((8, NPAIR, 128), BF16)
        rwk = np.zeros((128, 3, NPAIR, 128), BF16)
        rbk = np.zeros((128, 3, NPAIR), np.float32)
        w2k = np.zeros((128, NPAIR, 32), BF16)
        b2k = np.zeros((128, NSTACK), np.float32)
        tauk = np.zeros((128, NSTACK), np.float32)
        # garbage partitions keep tau=0 so exp(0)=1 stays finite
        embk = np.zeros((128, NSTACK, 256), BF16)
        for pr in range(NPAIR):
            fa, fb = f0 + 2 * pr, f0 + 2 * pr + 1
            xpk[pr, 0:4] = xT[fa]
            xpk[pr, 4:8] = xT[fb]
            w1k[0:4, pr, 0:64] = w1a[fa]
            w1k[4:8, pr, 64:128] = w1a[fb]
            for l in range(3):
                rwk[0:64, l, pr, 0:64] = rws[l][fa] + eye
                rwk[64:128, l, pr, 64:128] = rws[l][fb] + eye
                rbk[0:64, l, pr] = rbs[l][fa]
                rbk[64:128, l, pr] = rbs[l][fb]
            w2k[0:64, pr, 0:8] = w2[fa]
            w2k[64:128, pr, 8:16] = w2[fb]
            s, jj = pr // 4, pr % 4
            for fi, ff in ((0, fa), (1, fb)):
                rows = slice(32 * jj + 8 * fi, 32 * jj + 8 * fi + 8)
                b2k[rows, s] = b2[ff]
                tauk[rows, s] = tau[ff]
                embk[rows, s, 128 * fi : 128 * fi + 128] = emb[ff]
        # sum-over-k stationary with broadcast to all 128 rows; garbage
        # partitions duplicate the pair's second feature so values stay sane.
        ob = np.zeros((128, 128), BF16)
        for jj in range(4):
            for g in range(4):
                src = 32 * jj + 8 * min(g, 1)
                ob[src : src + 8, 32 * jj + 8 * g : 32 * jj + 8 * g + 8] = 1
        for s in range(NSTACK):
            for cc in range(2):
                for jj in range(4):
                    xq[s, :, cc, jj, :] = xpk[4 * s + jj][:, cc * HB : (cc + 1) * HB]
        m = {
            "xp": xq.reshape(NSTACK, 8, 2 * 4 * HB),
            "w1p": w1k.reshape(8, NPAIR * 128),
            "rwp": rwk.reshape(128, 3 * NPAIR * 128),
            "rbp": rbk.reshape(128, 3 * NPAIR),
            "w2p": w2k.reshape(128, NPAIR * 32),
            "b2s": b2k,
            "taus": tauk,
            "embs": embk.reshape(128, NSTACK * 256),
            "onesbd": ob,
        }
        in_maps.append(m)
    return in_maps


def _get_compiled():
    global _compiled
    if _compiled is None:
        _compiled = _build_bass()
    return _compiled


def run_on_hw(in_maps, trace=False):
    from concourse import bass_utils

    nc = _get_compiled()
    res = bass_utils.run_bass_kernel_spmd(
        nc, in_maps, core_ids=list(range(NCORES)), trace=trace
    )
    return res


def kernel(**inputs):
    in_maps = _host_pack(inputs)
    res = run_on_hw(in_maps, trace=False)
    outs = [np.asarray(res.results[c]["out"], dtype=np.float32) for c in range(NCORES)]
    return np.concatenate(outs, axis=1)


# revision 28
# speedup vs baseline: 1.2444x; 1.2444x over previous
"""AutoDisBucketEncoder Trainium2 kernel (8 NeuronCores, feature-sharded).

Math (per feature f, batch b):
  h = lrelu(x_aug @ w1_aug)            # bias folded via ones column
  h = lrelu(h @ (rw_l + I) + rb_l)     # x3, residual folded into weights
  z = lrelu(h @ w2 + b2)
  e = exp(z * tau)
  out = (e / sum_k e) @ emb

Layout: features sharded 32/core; each core packs 2 features per 128
partitions (block-diagonal weights), streams the full 2048 batch as the
matmul moving dim.  Softmax runs in [k, b] layout; the sum-over-k and its
broadcast back to 128 partitions are done by one ones-block matmul.

Perf structure (v2):
  * leaky relu runs as Prelu (parametric_relu) -- it lives in the same ACT
    table set as Exp and Copy, so the scalar engine never reloads tables.
  * every PSUM->SBUF evacuation (h evictions, output casts, z epilogue) is
    greedily load-balanced between the scalar (ACT) and vector (DVE)
    engines; DVE uses a custom fused max(x+b,(x+b)*a) op.
  * z matmuls issue q-major so the four col-tiled (M=32) matmuls run
    concurrently in the PE array; emb matmuls issue row-tiled (K=16 at
    row 32j) so four run concurrently per 128-batch block, producing one
    [128,1024] PSUM tile that maps to a contiguous 2KB-per-row output DMA.
"""

import sys

sys.path.insert(0, "/opt/trn_rl_repo")

import numpy as np
import ml_dtypes
from contextlib import ExitStack

BF16 = ml_dtypes.bfloat16
B, F, D, K, E = 2048, 256, 64, 8, 128
NCORES = 8
FC = F // NCORES          # 32 features per core
NPAIR = FC // 2           # 16
NSTACK = NPAIR // 4       # 4 stacks of 4 pairs
NEG = 0.01                # leaky slope
HB = B // 2               # 1024 batch half-chunk (2 PSUM banks in f32)

# build-time engine cost estimates (us) for the greedy ACT/DVE balancer
ACT_COST = 1.12
DVE_COST = 1.26
CP_ACT_COST = 1.12
CP_DVE_COST = 1.22
GPSIMD_TAIL = True  # run softmax mul on the idle Pool engine

_compiled = None
SIM_SAFE = False  # substitute Relu for Prelu so CoreSim can execute


def _register_leaky_bias():
    import numpy as np
    from concourse.dve_spec import Spec, Src0, C0, C1, maxx, lower
    from concourse.dve_ops import (
        DveOp, DveOpSpec, OPS, CUSTOM_DVE_SPECS, _SUB_OPCODE_FOR_NAME,
        _CUSTOM_DVE_ROW_BASE, has_src1,
    )

    if "LEAKY_BIAS_ANT" in CUSTOM_DVE_SPECS:
        return next(o for o in OPS if o.name == "LEAKY_BIAS_ANT")
    spec = Spec(
        body=maxx(Src0 + C0, (Src0 + C0) * C1),
        reference=lambda in0, in1, s0, s1, imm2: np.maximum(
            in0 + s0, (in0 + s0) * s1
        ).astype(np.float32),
    )
    row = _CUSTOM_DVE_ROW_BASE + len(OPS)
    shas = {}
    for ver in ("v3", "v4"):
        uops = lower(spec, ver=ver)
        shas[ver] = DveOpSpec(
            name="LEAKY_BIAS_ANT", opcode=row, uops=uops, rd1_en=has_src1(spec)
        ).sha(ver)
    op = DveOp("LEAKY_BIAS_ANT", spec, subdim=False, uops_sha=shas)
    OPS.append(op)
    CUSTOM_DVE_SPECS[op.name] = spec
    _SUB_OPCODE_FOR_NAME[op.name] = row
    return op


def _build_bass():
    import concourse.bass as bass  # noqa: F401
    import concourse.mybir as mybir
    import concourse.tile as tile
    from concourse import bacc

    LEAKY_OP = _register_leaky_bias()

    dt = mybir.dt
    AF = mybir.ActivationFunctionType
    PRELU = AF.Relu if SIM_SAFE else AF.Prelu

    nc = bacc.Bacc("TRN2", target_bir_lowering=False, debug=False)

    xp = nc.dram_tensor("xp", [NSTACK, 8, 2 * 4 * HB], dt.bfloat16, kind="ExternalInput").ap()
    w1p = nc.dram_tensor("w1p", [8, NPAIR * 128], dt.bfloat16, kind="ExternalInput").ap()
    rwp = nc.dram_tensor("rwp", [128, 3 * NPAIR * 128], dt.bfloat16, kind="ExternalInput").ap()
    rbp = nc.dram_tensor("rbp", [128, 3 * NPAIR], dt.float32, kind="ExternalInput").ap()
    w2p = nc.dram_tensor("w2p", [128, NPAIR * 32], dt.bfloat16, kind="ExternalInput").ap()
    b2s = nc.dram_tensor("b2s", [128, NSTACK], dt.float32, kind="ExternalInput").ap()
    taus = nc.dram_tensor("taus", [128, NSTACK], dt.float32, kind="ExternalInput").ap()
    onesbd = nc.dram_tensor("onesbd", [128, 128], dt.bfloat16, kind="ExternalInput").ap()
    embs = nc.dram_tensor("embs", [128, NSTACK * 256], dt.bfloat16, kind="ExternalInput").ap()
    out = nc.dram_tensor("out", [B, FC * E], dt.bfloat16, kind="ExternalOutput").ap()

    with tile.TileContext(nc) as tc, ExitStack() as ctx:
        const = ctx.enter_context(tc.tile_pool(name="const", bufs=1))
        xpool = ctx.enter_context(tc.tile_pool(name="xpool", bufs=3))
        hpool = ctx.enter_context(tc.tile_pool(name="hpool", bufs=8))
        tpool = ctx.enter_context(tc.tile_pool(name="tpool", bufs=3))
        epool = ctx.enter_context(tc.tile_pool(name="epool", bufs=2))
        rpool = ctx.enter_context(tc.tile_pool(name="rpool", bufs=2))
        opool = ctx.enter_context(tc.tile_pool(name="opool", bufs=4))
        h_ps = ctx.enter_context(tc.tile_pool(name="h_ps", bufs=3, space="PSUM"))
        zs_ps = ctx.enter_context(tc.tile_pool(name="zs_ps", bufs=1, space="PSUM"))

        # ---- constants into SBUF, ordered by first use so the single DMA
        # queue does not delay the first matmuls: w1 + first-chunk x go
        # first, the heavy rw stack next, everything else after ----
        xtiles = {}

        def fetch_x(s_, c_):
            if (s_, c_) in xtiles:
                return
            x_sb = xpool.tile([8, 4 * HB], dt.bfloat16, tag="x", name=f"x{s_}_{c_}")
            nc.sync.dma_start(
                out=x_sb, in_=xp[s_][:, c_ * 4 * HB : (c_ + 1) * 4 * HB]
            )
            xtiles[(s_, c_)] = x_sb

        w1_sb = const.tile([8, NPAIR * 128], dt.bfloat16)
        nc.sync.dma_start(out=w1_sb, in_=w1p)
        fetch_x(0, 0)
        rb_sb = const.tile([128, 3 * NPAIR], dt.float32)
        nc.sync.dma_start(out=rb_sb, in_=rbp)
        rw_sb = const.tile([128, 3 * NPAIR * 128], dt.bfloat16)
        for l in range(3):
            nc.sync.dma_start(
                out=rw_sb[:, l * NPAIR * 128 : (l + 1) * NPAIR * 128],
                in_=rwp[:, l * NPAIR * 128 : (l + 1) * NPAIR * 128],
            )
        fetch_x(0, 1)
        w2_sb = const.tile([128, NPAIR * 32], dt.bfloat16)
        nc.sync.dma_start(out=w2_sb, in_=w2p)
        b2_sb = const.tile([128, NSTACK], dt.float32)
        nc.sync.dma_start(out=b2_sb, in_=b2s)
        tau_sb = const.tile([128, NSTACK], dt.float32)
        nc.sync.dma_start(out=tau_sb, in_=taus)
        ones_sb = const.tile([128, 128], dt.bfloat16)
        nc.sync.dma_start(out=ones_sb, in_=onesbd)
        emb_sb = const.tile([128, NSTACK * 256], dt.bfloat16)
        nc.sync.dma_start(out=emb_sb, in_=embs)

        # out[b, fc*E] viewed as [qb(4), s(4), p(128), i(4), j(4), e(256)]
        out_r = out.rearrange("(qb i p) (s j e) -> qb s p i j e", p=128, i=4, j=4, e=256)

        # ---- greedy ACT/DVE load balancer for PSUM evacuations ----
        load = {"act": 0.0, "dve": 0.0}

        def evict_h(h, ph, rb_ap):
            """psum -> sbuf bf16 with (optional bias add and) leaky relu."""
            if load["act"] + ACT_COST <= load["dve"] + DVE_COST:
                load["act"] += ACT_COST
                if rb_ap is None:
                    nc.scalar.activation(h, ph, PRELU, alpha=NEG)
                else:
                    nc.scalar.activation(h, ph, PRELU, bias=rb_ap, alpha=NEG)
            else:
                load["dve"] += DVE_COST
                nc.vector._custom_dve(
                    LEAKY_OP,
                    out=h,
                    in0=ph,
                    s0=0.0 if rb_ap is None else rb_ap,
                    s1=NEG,
                )

        def copy_out(ob, po):
            """psum f32 -> sbuf bf16 plain cast."""
            if load["act"] + CP_ACT_COST <= load["dve"] + CP_DVE_COST:
                load["act"] += CP_ACT_COST
                nc.scalar.copy(ob, po)
            else:
                load["dve"] += CP_DVE_COST
                nc.vector.tensor_copy(ob, po)

        pending_zt = []    # deferred z-epilogue head (t1 + exp)
        pending_sum = []   # deferred sum-over-k matmuls
        pending_tail = []  # deferred softmax tail (recip/cast/mul)
        pending_emb = []   # emb batch-block closures

        def emit(lst, n=99):
            for _ in range(min(n, len(lst))):
                lst.pop(0)()

        chunks = [(s, c) for s in range(NSTACK) for c in range(2)]
        e_sbs = {}
        for ci, (s, c) in enumerate(chunks):
            if c == 0:
                e_sbs[s] = (
                    epool.tile([128, B], dt.bfloat16, tag="e", name=f"e{s}"),
                    epool.tile([128, B], dt.bfloat16, tag="en", name=f"en{s}"),
                )
            e_sb, en_sb = e_sbs[s]
            if True:
                # ---- h pipeline: pairs interleaved layer-step-wise; the
                # previous chunk's softmax/emb work drips in between steps ----
                emit(pending_zt)
                fetch_x(s, c)
                x_sb = xtiles.pop((s, c))
                hs = [x_sb[:, j * HB : (j + 1) * HB] for j in range(4)]
                if ci + 1 < len(chunks):
                    fetch_x(*chunks[ci + 1])
                for step in range(4):  # 0: L1, 1-3: residual layers
                    for j in range(4):
                        p = 4 * s + j
                        ph = h_ps.tile(
                            [128, HB], dt.float32, tag="h", name=f"ph{p}_{c}_{step}"
                        )
                        if step == 0:
                            wsl = w1_sb[:, p * 128 : (p + 1) * 128]
                            rb_ap = None
                        else:
                            l = step - 1
                            wsl = rw_sb[
                                :, (l * NPAIR + p) * 128 : (l * NPAIR + p + 1) * 128
                            ]
                            rb_ap = rb_sb[:, l * NPAIR + p : l * NPAIR + p + 1]
                        for q in range(2):
                            nc.tensor.matmul(
                                ph[:, q * 512 : (q + 1) * 512],
                                wsl,
                                hs[j][:, q * 512 : (q + 1) * 512],
                                start=True,
                                stop=True,
                            )
                        h2 = hpool.tile(
                            [128, HB], dt.bfloat16, tag="h", name=f"h{p}_{c}_{step}"
                        )
                        evict_h(h2, ph, rb_ap)
                        hs[j] = h2
                    if step == 0:
                        emit(pending_sum)
                    elif step == 1:
                        emit(pending_tail)
                    else:
                        emit(pending_emb, 4)
                # z matmuls: q-major so the 4 col-tiled (M=32) mms overlap
                pz = zs_ps.tile([128, HB], dt.float32, tag="zs", name=f"pz{s}_{c}")
                for q in range(2):
                    for j in range(4):
                        p = 4 * s + j
                        nc.tensor.matmul(
                            pz[32 * j : 32 * j + 32, q * 512 : (q + 1) * 512],
                            w2_sb[:, p * 32 : (p + 1) * 32],
                            hs[j][:, q * 512 : (q + 1) * 512],
                            start=True,
                            stop=True,
                            tile_position=(0, 32 * j),
                        )
                emit(pending_emb)

                def make_z(s_, c_, pz_ref, e_ref, en_ref):
                    def z_head():
                        t1 = tpool.tile(
                            [128, HB], dt.float32, tag="zt", name=f"t1_{s_}_{c_}"
                        )
                        b2_ap = b2_sb[:, s_ : s_ + 1]
                        if load["act"] + ACT_COST <= load["dve"] + DVE_COST:
                            load["act"] += ACT_COST
                            nc.scalar.activation(
                                t1, pz_ref, PRELU, bias=b2_ap, alpha=NEG
                            )
                        else:
                            load["dve"] += DVE_COST
                            nc.vector._custom_dve(
                                LEAKY_OP, out=t1, in0=pz_ref, s0=b2_ap, s1=NEG
                            )
                        ev = e_ref[:, c_ * HB : (c_ + 1) * HB]
                        load["act"] += ACT_COST
                        nc.scalar.activation(
                            ev, t1, AF.Exp, scale=tau_sb[:, s_ : s_ + 1]
                        )

                        def z_sum():
                            ps_sum = zs_ps.tile(
                                [128, HB], dt.float32, tag="zs", name=f"psum{s_}_{c_}"
                            )
                            for q in range(2):
                                nc.tensor.matmul(
                                    ps_sum[:, q * 512 : (q + 1) * 512],
                                    ones_sb,
                                    ev[:, q * 512 : (q + 1) * 512],
                                    start=True,
                                    stop=True,
                                )

                            def tail():
                                rcf = rpool.tile(
                                    [128, HB], dt.float32, tag="rcf",
                                    name=f"rcf{s_}_{c_}",
                                )
                                load["dve"] += DVE_COST
                                nc.vector.reciprocal_approx_fast(out=rcf, in_=ps_sum)
                                rc = rpool.tile(
                                    [128, HB], dt.bfloat16, tag="rc",
                                    name=f"rc{s_}_{c_}",
                                )
                                en_full = en_ref[:, c_ * HB : (c_ + 1) * HB]
                                load["dve"] += 0.83
                                nc.vector.tensor_copy(rc, rcf)
                                for hh in range(2):
                                    if GPSIMD_TAIL:
                                        nc.gpsimd.tensor_mul(
                                            en_full[:, hh * 512 : (hh + 1) * 512],
                                            ev[:, hh * 512 : (hh + 1) * 512],
                                            rc[:, hh * 512 : (hh + 1) * 512],
                                        )
                                    else:
                                        load["dve"] += 0.45
                                        nc.vector.tensor_mul(
                                            en_full[:, hh * 512 : (hh + 1) * 512],
                                            ev[:, hh * 512 : (hh + 1) * 512],
                                            rc[:, hh * 512 : (hh + 1) * 512],
                                        )

                            pending_tail.append(tail)

                        pending_sum.append(z_sum)

                    return z_head

                pending_zt.append(make_z(s, c, pz, e_sb, en_sb))

                def make_group(s_, qb_, j_, en_ref):
                    def emit_group():
                        po = h_ps.tile(
                            [128, 4, 256],
                            dt.float32,
                            tag="h",
                            name=f"po{s_}_{qb_}_{j_}",
                        )
                        for i in range(4):
                            bc2 = qb_ * 4 + i
                            nc.tensor.matmul(
                                po[:, i, :],
                                en_ref[
                                    32 * j_ : 32 * j_ + 16, bc2 * 128 : (bc2 + 1) * 128
                                ],
                                emb_sb[32 * j_ : 32 * j_ + 16, s_ * 256 : (s_ + 1) * 256],
                                start=True,
                                stop=True,
                                tile_position=(32 * j_, 0),
                            )
                        ob = opool.tile(
                            [128, 4, 256], dt.bfloat16, tag="o",
                            name=f"ob{s_}_{qb_}_{j_}",
                        )
                        copy_out(ob, po)
                        nc.sync.dma_start(out=out_r[qb_, s_][:, :, j_, :], in_=ob)

                    return emit_group

                for qb in (2 * c, 2 * c + 1):
                    for j in range(4):
                        pending_emb.append(make_group(s, qb, j, en_sb))
        emit(pending_zt)
        emit(pending_sum)
        emit(pending_tail)
        emit(pending_emb)

    nc.compile()
    return nc


def _host_pack(inputs):
    """Pack full f32 inputs into per-core bf16 device arrays."""
    x = np.ascontiguousarray(inputs["x"], dtype=np.float32)
    w1 = np.asarray(inputs["w1"], dtype=np.float32)
    b1 = np.asarray(inputs["b1"], dtype=np.float32)
    w2 = np.asarray(inputs["w2"], dtype=np.float32)
    b2 = np.asarray(inputs["b2"], dtype=np.float32)
    tau = np.asarray(inputs["tau"], dtype=np.float32)
    emb = np.asarray(inputs["emb"], dtype=np.float32)
    rws = [np.asarray(inputs[f"rw{l}"], dtype=np.float32) for l in range(3)]
    rbs = [np.asarray(inputs[f"rb{l}"], dtype=np.float32) for l in range(3)]

    eye = np.eye(D, dtype=np.float32)
    xT = np.concatenate([x, np.ones((B, F, 1), np.float32)], axis=2)
    xT = np.ascontiguousarray(xT.transpose(1, 2, 0))  # [F, 4, B]
    w1a = np.concatenate([w1, b1[:, None, :]], axis=1)  # [F, 4, D]

    in_maps = []
    for cidx in range(NCORES):
        f0 = cidx * FC
        xpk = np.zeros((NPAIR, 8, B), BF16)
        xq = np.zeros((NSTACK, 8, 2, 4, HB), BF16)
        w1k = np.zeros((8, NPAIR, 128), BF16)
        rwk = np.zeros((128, 3, NPAIR, 128), BF16)
        rbk = np.zeros((128, 3, NPAIR), np.float32)
        w2k = np.zeros((128, NPAIR, 32), BF16)
        b2k = np.zeros((128, NSTACK), np.float32)
        tauk = np.zeros((128, NSTACK), np.float32)
        # garbage partitions keep tau=0 so exp(0)=1 stays finite
        embk = np.zeros((128, NSTACK, 256), BF16)
        for pr in range(NPAIR):
            fa, fb = f0 + 2 * pr, f0 + 2 * pr + 1
            xpk[pr, 0:4] = xT[fa]
            xpk[pr, 4:8] = xT[fb]
            w1k[0:4, pr, 0:64] = w1a[fa]
            w1k[4:8, pr, 64:128] = w1a[fb]
            for l in range(3):
                rwk[0:64, l, pr, 0:64] = rws[l][fa] + eye
                rwk[64:128, l, pr, 64:128] = rws[l][fb] + eye
                rbk[0:64, l, pr] = rbs[l][fa]
                rbk[64:128, l, pr] = rbs[l][fb]
            w2k[0:64, pr, 0:8] = w2[fa]
            w2k[64:128, pr, 8:16] = w2[fb]
            s, jj = pr // 4, pr % 4
            for fi, ff in ((0, fa), (1, fb)):
                rows = slice(32 * jj + 8 * fi, 32 * jj + 8 * fi + 8)
                b2k[rows, s] = b2[ff]
                tauk[rows, s] = tau[ff]
                embk[rows, s, 128 * fi : 128 * fi + 128] = emb[ff]
        # sum-over-k stationary with broadcast to all 128 rows; garbage
        # partitions duplicate the pair's second feature so values stay sane.
        ob = np.zeros((128, 128), BF16)
        for jj in range(4):
            for g in range(4):
                src = 32 * jj + 8 * min(g, 1)
                ob[src : src + 8, 32 * jj + 8 * g : 32 * jj + 8 * g + 8] = 1
        for s in range(NSTACK):
            for cc in range(2):
                for jj in range(4):
                    xq[s, :, cc, jj, :] = xpk[4 * s + jj][:, cc * HB : (cc + 1) * HB]
        m = {
            "xp": xq.reshape(NSTACK, 8, 2 * 4 * HB),
            "w1p": w1k.reshape(8, NPAIR * 128),
            "rwp": rwk.reshape(128, 3 * NPAIR * 128),
            "rbp": rbk.reshape(128, 3 * NPAIR),
            "w2p": w2k.reshape(128, NPAIR * 32),
            "b2s": b2k,
            "taus": tauk,
            "embs": embk.reshape(128, NSTACK * 256),
            "onesbd": ob,
        }
        in_maps.append(m)
    return in_maps


def _get_compiled():
    global _compiled
    if _compiled is None:
        _compiled = _build_bass()
    return _compiled


def run_on_hw(in_maps, trace=False):
    from concourse import bass_utils

    nc = _get_compiled()
    res = bass_utils.run_bass_kernel_spmd(
        nc, in_maps, core_ids=list(range(NCORES)), trace=trace
    )
    return res


def kernel(**inputs):
    in_maps = _host_pack(inputs)
    res = run_on_hw(in_maps, trace=False)
    outs = [np.asarray(res.results[c]["out"], dtype=np.float32) for c in range(NCORES)]
    return np.concatenate(outs, axis=1)


# revision 31
# speedup vs baseline: 1.2747x; 1.0244x over previous
"""AutoDisBucketEncoder Trainium2 kernel (8 NeuronCores, feature-sharded).

Math (per feature f, batch b):
  h = lrelu(x_aug @ w1_aug)            # bias folded via ones column
  h = lrelu(h @ (rw_l + I) + rb_l)     # x3, residual folded into weights
  z = lrelu(h @ w2 + b2)
  e = exp(z * tau)
  out = (e / sum_k e) @ emb

Layout: features sharded 32/core; each core packs 2 features per 128
partitions (block-diagonal weights), streams the full 2048 batch as the
matmul moving dim.  Softmax runs in [k, b] layout; the sum-over-k and its
broadcast back to 128 partitions are done by one ones-block matmul.

Perf structure (v2):
  * leaky relu runs as Prelu (parametric_relu) -- it lives in the same ACT
    table set as Exp and Copy, so the scalar engine never reloads tables.
  * every PSUM->SBUF evacuation (h evictions, output casts, z epilogue) is
    greedily load-balanced between the scalar (ACT) and vector (DVE)
    engines; DVE uses a custom fused max(x+b,(x+b)*a) op.
  * z matmuls issue q-major so the four col-tiled (M=32) matmuls run
    concurrently in the PE array; emb matmuls issue row-tiled (K=16 at
    row 32j) so four run concurrently per 128-batch block, producing one
    [128,1024] PSUM tile that maps to a contiguous 2KB-per-row output DMA.
"""

import sys

sys.path.insert(0, "/opt/trn_rl_repo")

import numpy as np
import ml_dtypes
from contextlib import ExitStack

BF16 = ml_dtypes.bfloat16
B, F, D, K, E = 2048, 256, 64, 8, 128
NCORES = 8
FC = F // NCORES          # 32 features per core
NPAIR = FC // 2           # 16
NSTACK = NPAIR // 4       # 4 stacks of 4 pairs
NEG = 0.01                # leaky slope
HB = B // 2               # 1024 batch half-chunk (2 PSUM banks in f32)

# build-time engine cost estimates (us) for the greedy ACT/DVE balancer
ACT_COST = 1.12
DVE_COST = 1.26
CP_ACT_COST = 1.12
CP_DVE_COST = 1.22
GPSIMD_TAIL = True  # run softmax mul on the idle Pool engine

_compiled = None
SIM_SAFE = False  # substitute Relu for Prelu so CoreSim can execute


def _register_leaky_bias():
    import numpy as np
    from concourse.dve_spec import Spec, Src0, C0, C1, maxx, lower
    from concourse.dve_ops import (
        DveOp, DveOpSpec, OPS, CUSTOM_DVE_SPECS, _SUB_OPCODE_FOR_NAME,
        _CUSTOM_DVE_ROW_BASE, has_src1,
    )

    if "LEAKY_BIAS_ANT" in CUSTOM_DVE_SPECS:
        return next(o for o in OPS if o.name == "LEAKY_BIAS_ANT")
    spec = Spec(
        body=maxx(Src0 + C0, (Src0 + C0) * C1),
        reference=lambda in0, in1, s0, s1, imm2: np.maximum(
            in0 + s0, (in0 + s0) * s1
        ).astype(np.float32),
    )
    row = _CUSTOM_DVE_ROW_BASE + len(OPS)
    shas = {}
    for ver in ("v3", "v4"):
        uops = lower(spec, ver=ver)
        shas[ver] = DveOpSpec(
            name="LEAKY_BIAS_ANT", opcode=row, uops=uops, rd1_en=has_src1(spec)
        ).sha(ver)
    op = DveOp("LEAKY_BIAS_ANT", spec, subdim=False, uops_sha=shas)
    OPS.append(op)
    CUSTOM_DVE_SPECS[op.name] = spec
    _SUB_OPCODE_FOR_NAME[op.name] = row
    return op


def _build_bass():
    import concourse.bass as bass  # noqa: F401
    import concourse.mybir as mybir
    import concourse.tile as tile
    from concourse import bacc

    LEAKY_OP = _register_leaky_bias()

    dt = mybir.dt
    AF = mybir.ActivationFunctionType
    PRELU = AF.Relu if SIM_SAFE else AF.Prelu

    nc = bacc.Bacc("TRN2", target_bir_lowering=False, debug=False)

    xp = nc.dram_tensor("xp", [NSTACK, 8, 2 * 4 * HB], dt.bfloat16, kind="ExternalInput").ap()
    w1p = nc.dram_tensor("w1p", [8, NPAIR * 128], dt.bfloat16, kind="ExternalInput").ap()
    rwp = nc.dram_tensor("rwp", [128, 3 * NPAIR * 128], dt.bfloat16, kind="ExternalInput").ap()
    rbp = nc.dram_tensor("rbp", [128, 3 * NPAIR], dt.float32, kind="ExternalInput").ap()
    w2p = nc.dram_tensor("w2p", [128, NPAIR * 32], dt.bfloat16, kind="ExternalInput").ap()
    b2s = nc.dram_tensor("b2s", [128, NSTACK], dt.float32, kind="ExternalInput").ap()
    taus = nc.dram_tensor("taus", [128, NSTACK], dt.float32, kind="ExternalInput").ap()
    onesbd = nc.dram_tensor("onesbd", [128, 128], dt.bfloat16, kind="ExternalInput").ap()
    embs = nc.dram_tensor("embs", [128, NSTACK * 256], dt.bfloat16, kind="ExternalInput").ap()
    out = nc.dram_tensor("out", [B, FC * E], dt.bfloat16, kind="ExternalOutput").ap()

    with tile.TileContext(nc) as tc, ExitStack() as ctx:
        const = ctx.enter_context(tc.tile_pool(name="const", bufs=1))
        xpool = ctx.enter_context(tc.tile_pool(name="xpool", bufs=3))
        hpool = ctx.enter_context(tc.tile_pool(name="hpool", bufs=8))
        tpool = ctx.enter_context(tc.tile_pool(name="tpool", bufs=3))
        epool = ctx.enter_context(tc.tile_pool(name="epool", bufs=2))
        rpool = ctx.enter_context(tc.tile_pool(name="rpool", bufs=2))
        opool = ctx.enter_context(tc.tile_pool(name="opool", bufs=4))
        h_ps = ctx.enter_context(tc.tile_pool(name="h_ps", bufs=3, space="PSUM"))
        zs_ps = ctx.enter_context(tc.tile_pool(name="zs_ps", bufs=1, space="PSUM"))

        # ---- constants into SBUF, ordered by first use so the single DMA
        # queue does not delay the first matmuls: w1 + first-chunk x go
        # first, the heavy rw stack next, everything else after ----
        xtiles = {}

        def fetch_x(s_, c_):
            if (s_, c_) in xtiles:
                return
            x_sb = xpool.tile([8, 4 * HB], dt.bfloat16, tag="x", name=f"x{s_}_{c_}")
            nc.sync.dma_start(
                out=x_sb, in_=xp[s_][:, c_ * 4 * HB : (c_ + 1) * 4 * HB]
            )
            xtiles[(s_, c_)] = x_sb

        w1_sb = const.tile([8, NPAIR * 128], dt.bfloat16)
        nc.sync.dma_start(out=w1_sb, in_=w1p)
        fetch_x(0, 0)
        rb_sb = const.tile([128, 3 * NPAIR], dt.float32)
        nc.sync.dma_start(out=rb_sb, in_=rbp)
        rw_sb = const.tile([128, 3 * NPAIR * 128], dt.bfloat16)
        for l in range(3):
            nc.sync.dma_start(
                out=rw_sb[:, l * NPAIR * 128 : (l + 1) * NPAIR * 128],
                in_=rwp[:, l * NPAIR * 128 : (l + 1) * NPAIR * 128],
            )
        fetch_x(0, 1)
        w2_sb = const.tile([128, NPAIR * 32], dt.bfloat16)
        nc.sync.dma_start(out=w2_sb, in_=w2p)
        b2_sb = const.tile([128, NSTACK], dt.float32)
        nc.sync.dma_start(out=b2_sb, in_=b2s)
        tau_sb = const.tile([128, NSTACK], dt.float32)
        nc.sync.dma_start(out=tau_sb, in_=taus)
        ones_sb = const.tile([128, 128], dt.bfloat16)
        nc.sync.dma_start(out=ones_sb, in_=onesbd)
        emb_sb = const.tile([128, NSTACK * 256], dt.bfloat16)
        nc.sync.dma_start(out=emb_sb, in_=embs)

        # out[b, fc*E] viewed as [qb(4), s(4), p(128), i(4), j(4), e(256)]
        out_r = out.rearrange("(qb i p) (s j e) -> qb s p i j e", p=128, i=4, j=4, e=256)

        # ---- greedy ACT/DVE load balancer for PSUM evacuations ----
        load = {"act": 0.0, "dve": 0.0}

        def evict_h(h, ph, rb_ap):
            """psum -> sbuf bf16 with (optional bias add and) leaky relu."""
            if load["act"] + ACT_COST <= load["dve"] + DVE_COST:
                load["act"] += ACT_COST
                if rb_ap is None:
                    nc.scalar.activation(h, ph, PRELU, alpha=NEG)
                else:
                    nc.scalar.activation(h, ph, PRELU, bias=rb_ap, alpha=NEG)
            else:
                load["dve"] += DVE_COST
                nc.vector._custom_dve(
                    LEAKY_OP,
                    out=h,
                    in0=ph,
                    s0=0.0 if rb_ap is None else rb_ap,
                    s1=NEG,
                )

        def copy_out(ob, po):
            """psum f32 -> sbuf bf16 plain cast."""
            if load["act"] + CP_ACT_COST <= load["dve"] + CP_DVE_COST:
                load["act"] += CP_ACT_COST
                nc.scalar.copy(ob, po)
            else:
                load["dve"] += CP_DVE_COST
                nc.vector.tensor_copy(ob, po)

        pending_zt = []    # deferred z-epilogue head (t1 + exp)
        pending_sum = []   # deferred sum-over-k matmuls
        pending_tail = []  # deferred softmax tail (recip/cast/mul)
        pending_emb = []   # emb batch-block closures

        def emit(lst, n=99):
            for _ in range(min(n, len(lst))):
                lst.pop(0)()

        chunks = [(s, c) for s in range(NSTACK) for c in range(2)]
        e_sbs = {}
        for ci, (s, c) in enumerate(chunks):
            if c == 0:
                e_sbs[s] = (
                    epool.tile([128, B], dt.bfloat16, tag="e", name=f"e{s}"),
                    epool.tile([128, B], dt.bfloat16, tag="en", name=f"en{s}"),
                )
            e_sb, en_sb = e_sbs[s]
            if True:
                # ---- h pipeline: pairs interleaved layer-step-wise; the
                # previous chunk's softmax/emb work drips in between steps ----
                emit(pending_zt)
                fetch_x(s, c)
                x_sb = xtiles.pop((s, c))
                hs = [x_sb[:, j * HB : (j + 1) * HB] for j in range(4)]
                if ci + 1 < len(chunks):
                    fetch_x(*chunks[ci + 1])
                for step in range(4):  # 0: L1, 1-3: residual layers
                    for j in range(4):
                        p = 4 * s + j
                        ph = h_ps.tile(
                            [128, HB], dt.float32, tag="h", name=f"ph{p}_{c}_{step}"
                        )
                        if step == 0:
                            wsl = w1_sb[:, p * 128 : (p + 1) * 128]
                            rb_ap = None
                        else:
                            l = step - 1
                            wsl = rw_sb[
                                :, (l * NPAIR + p) * 128 : (l * NPAIR + p + 1) * 128
                            ]
                            rb_ap = rb_sb[:, l * NPAIR + p : l * NPAIR + p + 1]
                        for q in range(2):
                            nc.tensor.matmul(
                                ph[:, q * 512 : (q + 1) * 512],
                                wsl,
                                hs[j][:, q * 512 : (q + 1) * 512],
                                start=True,
                                stop=True,
                            )
                        h2 = hpool.tile(
                            [128, HB], dt.bfloat16, tag="h", name=f"h{p}_{c}_{step}"
                        )
                        evict_h(h2, ph, rb_ap)
                        hs[j] = h2
                    if step == 0:
                        emit(pending_sum)
                    elif step == 1:
                        emit(pending_tail)
                    else:
                        emit(pending_emb, 3)
                # z matmuls: q-major so the 4 col-tiled (M=32) mms overlap
                pz = zs_ps.tile([128, HB], dt.float32, tag="zs", name=f"pz{s}_{c}")
                for q in range(2):
                    for j in range(4):
                        p = 4 * s + j
                        nc.tensor.matmul(
                            pz[32 * j : 32 * j + 32, q * 512 : (q + 1) * 512],
                            w2_sb[:, p * 32 : (p + 1) * 32],
                            hs[j][:, q * 512 : (q + 1) * 512],
                            start=True,
                            stop=True,
                            tile_position=(0, 32 * j),
                        )
                emit(pending_emb)

                def make_z(s_, c_, pz_ref, e_ref, en_ref, split):
                    shared = {}
                    widths = [(0, 512), (512, 512)] if split else [(0, HB)]

                    def make_head(lo, w):
                        def z_head():
                            if "t1" not in shared:
                                shared["t1"] = tpool.tile(
                                    [128, HB], dt.float32, tag="zt",
                                    name=f"t1_{s_}_{c_}",
                                )
                            t1 = shared["t1"][:, lo : lo + w]
                            pzs = pz_ref[:, lo : lo + w]
                            b2_ap = b2_sb[:, s_ : s_ + 1]
                            if load["act"] + ACT_COST <= load["dve"] + DVE_COST:
                                load["act"] += ACT_COST
                                nc.scalar.activation(
                                    t1, pzs, PRELU, bias=b2_ap, alpha=NEG
                                )
                            else:
                                load["dve"] += DVE_COST
                                nc.vector._custom_dve(
                                    LEAKY_OP, out=t1, in0=pzs, s0=b2_ap, s1=NEG
                                )
                            ev = e_ref[:, c_ * HB + lo : c_ * HB + lo + w]
                            load["act"] += ACT_COST
                            nc.scalar.activation(
                                ev, t1, AF.Exp, scale=tau_sb[:, s_ : s_ + 1]
                            )

                            def z_sum():
                                if "ps" not in shared:
                                    shared["ps"] = zs_ps.tile(
                                        [128, HB], dt.float32, tag="zs",
                                        name=f"psum{s_}_{c_}",
                                    )
                                ps_sum = shared["ps"][:, lo : lo + w]
                                for q0 in range(0, w, 512):
                                    nc.tensor.matmul(
                                        ps_sum[:, q0 : q0 + 512],
                                        ones_sb,
                                        ev[:, q0 : q0 + 512],
                                        start=True,
                                        stop=True,
                                    )

                                def tail():
                                    if "rcf" not in shared:
                                        shared["rcf"] = rpool.tile(
                                            [128, HB], dt.float32, tag="rcf",
                                            name=f"rcf{s_}_{c_}",
                                        )
                                        shared["rc"] = rpool.tile(
                                            [128, HB], dt.bfloat16, tag="rc",
                                            name=f"rc{s_}_{c_}",
                                        )
                                    rcf = shared["rcf"][:, lo : lo + w]
                                    rc = shared["rc"][:, lo : lo + w]
                                    load["dve"] += DVE_COST
                                    nc.vector.reciprocal_approx_fast(out=rcf, in_=ps_sum)
                                    en_w = en_ref[:, c_ * HB + lo : c_ * HB + lo + w]
                                    load["dve"] += 0.83
                                    nc.vector.tensor_copy(rc, rcf)
                                    for hh in range(0, w, 512):
                                        if GPSIMD_TAIL:
                                            nc.gpsimd.tensor_mul(
                                                en_w[:, hh : hh + 512],
                                                ev[:, hh : hh + 512],
                                                rc[:, hh : hh + 512],
                                            )
                                        else:
                                            load["dve"] += 0.45
                                            nc.vector.tensor_mul(
                                                en_w[:, hh : hh + 512],
                                                ev[:, hh : hh + 512],
                                                rc[:, hh : hh + 512],
                                            )

                                pending_tail.append(tail)

                            pending_sum.append(z_sum)

                        return z_head

                    for lo, w in widths:
                        pending_zt.append(make_head(lo, w))

                make_z(s, c, pz, e_sb, en_sb, split=(ci == len(chunks) - 1))

                def make_group(s_, qb_, j_, en_ref):
                    def emit_group():
                        po = h_ps.tile(
                            [128, 4, 256],
                            dt.float32,
                            tag="h",
                            name=f"po{s_}_{qb_}_{j_}",
                        )
                        for i in range(4):
                            bc2 = qb_ * 4 + i
                            nc.tensor.matmul(
                                po[:, i, :],
                                en_ref[
                                    32 * j_ : 32 * j_ + 16, bc2 * 128 : (bc2 + 1) * 128
                                ],
                                emb_sb[32 * j_ : 32 * j_ + 16, s_ * 256 : (s_ + 1) * 256],
                                start=True,
                                stop=True,
                                tile_position=(32 * j_, 0),
                            )
                        ob = opool.tile(
                            [128, 4, 256], dt.bfloat16, tag="o",
                            name=f"ob{s_}_{qb_}_{j_}",
                        )
                        copy_out(ob, po)
                        nc.sync.dma_start(out=out_r[qb_, s_][:, :, j_, :], in_=ob)

                    return emit_group

                for qb in (2 * c, 2 * c + 1):
                    for j in range(4):
                        pending_emb.append(make_group(s, qb, j, en_sb))
        # final flush: the last chunk's z-chain was emitted in 512-wide
        # halves; interleave so the first half's softmax + emb groups run
        # while the second half's chain is still in flight
        emit(pending_zt, 1)
        emit(pending_sum, 1)
        emit(pending_zt, 1)
        emit(pending_tail, 1)
        emit(pending_sum, 1)
        emit(pending_emb, 4)
        emit(pending_tail, 1)
        emit(pending_zt)
        emit(pending_sum)
        emit(pending_tail)
        emit(pending_emb)

    nc.compile()
    return nc


def _host_pack(inputs):
    """Pack full f32 inputs into per-core bf16 device arrays."""
    x = np.ascontiguousarray(inputs["x"], dtype=np.float32)
    w1 = np.asarray(inputs["w1"], dtype=np.float32)
    b1 = np.asarray(inputs["b1"], dtype=np.float32)
    w2 = np.asarray(inputs["w2"], dtype=np.float32)
    b2 = np.asarray(inputs["b2"], dtype=np.float32)
    tau = np.asarray(inputs["tau"], dtype=np.float32)
    emb = np.asarray(inputs["emb"], dtype=np.float32)
    rws = [np.asarray(inputs[f"rw{l}"], dtype=np.float32) for l in range(3)]
    rbs = [np.asarray(inputs[f"rb{l}"], dtype=np.float32) for l in range(3)]

    eye = np.eye(D, dtype=np.float32)
    xT = np.concatenate([x, np.ones((B, F, 1), np.float32)], axis=2)
    xT = np.ascontiguousarray(xT.transpose(1, 2, 0))  # [F, 4, B]
    w1a = np.concatenate([w1, b1[:, None, :]], axis=1)  # [F, 4, D]

    in_maps = []
    for cidx in range(NCORES):
        f0 = cidx * FC
        xpk = np.zeros((NPAIR, 8, B), BF16)
        xq = np.zeros((NSTACK, 8, 2, 4, HB), BF16)
        w1k = np.zeros((8, NPAIR, 128), BF16)
        rwk = np.zeros((128, 3, NPAIR, 128), BF16)
        rbk = np.zeros((128, 3, NPAIR), np.float32)
        w2k = np.zeros((128, NPAIR, 32), BF16)
        b2k = np.zeros((128, NSTACK), np.float32)
        tauk = np.zeros((128, NSTACK), np.float32)
        # garbage partitions keep tau=0 so exp(0)=1 stays finite
        embk = np.zeros((128, NSTACK, 256), BF16)
        for pr in range(NPAIR):
            fa, fb = f0 + 2 * pr, f0 + 2 * pr + 1
            xpk[pr, 0:4] = xT[fa]
            xpk[pr, 4:8] = xT[fb]
            w1k[0:4, pr, 0:64] = w1a[fa]
            w1k[4:8, pr, 64:128] = w1a[fb]
            for l in range(3):
                rwk[0:64, l, pr, 0:64] = rws[l][fa] + eye
                rwk[64:128, l, pr, 64:128] = rws[l][fb] + eye
                rbk[0:64, l, pr] = rbs[l][fa]
                rbk[64:128, l, pr] = rbs[l][fb]
            w2k[0:64, pr, 0:8] = w2[fa]
            w2k[64:128, pr, 8:16] = w2[fb]
            s, jj = pr // 4, pr % 4
            for fi, ff in ((0, fa), (1, fb)):
                rows = slice(32 * jj + 8 * fi, 32 * jj + 8 * fi + 8)
                b2k[rows, s] = b2[ff]
                tauk[rows, s] = tau[ff]
                embk[rows, s, 128 * fi : 128 * fi + 128] = emb[ff]
        # sum-over-k stationary with broadcast to all 128 rows; garbage
        # partitions duplicate the pair's second feature so values stay sane.
        ob = np.zeros((128, 128), BF16)
        for jj in range(4):
            for g in range(4):
                src = 32 * jj + 8 * min(g, 1)
                ob[src : src + 8, 32 * jj + 8 * g : 32 * jj + 8 * g + 8] = 1
        for s in range(NSTACK):
            for cc in range(2):
                for jj in range(4):
                    xq[s, :, cc, jj, :] = xpk[4 * s + jj][:, cc * HB : (cc + 1) * HB]
        m = {
            "xp": xq.reshape(NSTACK, 8, 2 * 4 * HB),
            "w1p": w1k.reshape(8, NPAIR * 128),
            "rwp": rwk.reshape(128, 3 * NPAIR * 128),
            "rbp": rbk.reshape(128, 3 * NPAIR),
            "w2p": w2k.reshape(128, NPAIR * 32),
            "b2s": b2k,
            "taus": tauk,
            "embs": embk.reshape(128, NSTACK * 256),
            "onesbd": ob,
        }
        in_maps.append(m)
    return in_maps


def _get_compiled():
    global _compiled
    if _compiled is None:
        _compiled = _build_bass()
    return _compiled


def run_on_hw(in_maps, trace=False):
    from concourse import bass_utils

    nc = _get_compiled()
    res = bass_utils.run_bass_kernel_spmd(
        nc, in_maps, core_ids=list(range(NCORES)), trace=trace
    )
    return res


def kernel(**inputs):
    in_maps = _host_pack(inputs)
    res = run_on_hw(in_maps, trace=False)
    outs = [np.asarray(res.results[c]["out"], dtype=np.float32) for c in range(NCORES)]
    return np.concatenate(outs, axis=1)
